# revision 18
# baseline (speedup 1.0000x reference)
"""DeepBSDE forward-loss kernel for Trainium2 (8 NeuronCores, data-parallel).

Math (per sample b, 50 steps, dt=0.02):
    x_n = [t_n, y_n]                       (4 features)
    z_n = MLP_z(x_n)   (4->64->64->3, relu)
    q_n = MLP_q(x_n)   (4->64->64->1, relu)
    y_{n+1} = (1-dt) y_n + dt q_n + (0.2 + 0.1 tanh(y_n)) * sqrt(dt) * dW_n
    Y_final = Y0 - 0.5 dt sum_n q_n^2 + sum_n z_n . (sqrt(dt) dW_n)
    out = mean_b (Y_final - |y_final|^2)^2

Device layout (per core, B_loc = 16384 = 32 chunks x 512):
    every per-sample state lives in a [128, 512] SBUF tile:
      partition k        (k in 0..31)   : q-slot of chunk k
      partition 32+32i+k (i in 0..2)    : vector component i of chunk k
      free c                            : sample index b = k*512 + c
    The two MLPs are fused: hidden = [q-hidden(64) ; z-hidden(64)] = 128.
    Per step the PE streams: L1 (K=3), L2 (K=128), L3 (per-chunk sparse
    [128,128] stationaries accumulated into ONE packed psum bank), plus one
    broadcast matmul that replicates dt*q into the 3 component quarters.
"""

import sys
import os

for _p in ("/opt/trn_rl_repo", "/root/.axon_site/_ro/trn_rl_repo"):
    if os.path.isdir(_p) and _p not in sys.path:
        sys.path.insert(0, _p)

import numpy as np

DT = 0.02
SQRT_DT = float(np.sqrt(np.float32(DT)))
N_STEPS = 50
BATCH = 131072
DIM = 3
N_CORES = 8
B_LOC = BATCH // N_CORES          # 16384
CHUNKS = 32
FREE = B_LOC // CHUNKS            # 512

# dtype knobs.
#  - L1/BB/TT matmuls read fp32 state; run them as float32r (same 4-byte
#    storage, 1 cycle/row on the PE at moving size >= 256 vs 4 for fp32).
#  - hidden activations h1/h2 and the L2/L3 weights run in bf16: same PE
#    rate as f32r but half the ACT/DVE evacuation cost and half the
#    weight-load traffic.
MM_HID_F32 = os.environ.get("BSDE_HID_F32", "0") == "1"

# how many h2 evacuations run on the scalar (ACT) engine instead of DVE,
# to balance the two engines' per-step load.
ACT_H2 = int(os.environ.get("BSDE_ACT_H2", "4"))

# If set (by the timing harness), the device loop runs this many steps while
# all I/O shapes stay identical — lets wall-clock differencing isolate the
# per-step device time from RPC/transfer overhead.
LOOP_STEPS = None


def _np_f32(x):
    return np.ascontiguousarray(np.asarray(x, dtype=np.float32))


def prep_host(inputs):
    """Build all device-side arrays (numpy fp32) from the raw problem inputs."""
    i = {k: _np_f32(v) for k, v in inputs.items()}
    qW1, qb1 = i["qW1"], i["qb1"]
    qW2, qb2 = i["qW2"], i["qb2"]
    qW3, qb3 = i["qW3"], i["qb3"]
    zW1, zb1 = i["zW1"], i["zb1"]
    zW2, zb2 = i["zW2"], i["zb2"]
    zW3, zb3 = i["zW3"], i["zb3"]
    y0 = i["y0"]
    Y0 = float(i["Y0"].reshape(-1)[0])
    dW = i["dW"]

    W1cat = np.concatenate([qW1, zW1], axis=1)          # [4, 128]
    b1cat = np.concatenate([qb1, zb1])                  # [128]

    # L1 reads the packed y state directly: per-chunk sparse stationaries.
    # chunk k's component i lives at partition 32 + 32*i + k.
    W1S = np.zeros((CHUNKS, 128, 128), np.float32)
    for k in range(CHUNKS):
        for comp in range(3):
            W1S[k, 32 + 32 * comp + k, :] = W1cat[1 + comp, :]
    W1S = np.ascontiguousarray(W1S.transpose(1, 0, 2).reshape(128, CHUNKS * 128))

    # per-step bias for the L1 relu evacuation: c_n = t_n * W1cat[0] + b1cat
    ts = (np.arange(N_STEPS, dtype=np.float32) * np.float32(DT))
    CB = (ts[None, :] * W1cat[0][:, None] + b1cat[:, None]).astype(np.float32)  # [128, 50]

    W2 = np.zeros((128, 128), np.float32)
    W2[0:64, 0:64] = qW2
    W2[64:128, 64:128] = zW2
    B2 = b1cat * 0.0
    B2 = np.concatenate([qb2, zb2]).astype(np.float32).reshape(128, 1)

    # L3: per-chunk sparse stationaries [128, 32*128]
    W3S = np.zeros((CHUNKS, 128, 128), np.float32)
    for k in range(CHUNKS):
        W3S[k, 0:64, k] = DT * qW3[:, 0]
        for comp in range(3):
            W3S[k, 64:128, 32 + 32 * comp + k] = SQRT_DT * zW3[:, comp]
    W3S = np.ascontiguousarray(W3S.transpose(1, 0, 2).reshape(128, CHUNKS * 128))

    B3 = np.zeros((128, 1), np.float32)
    B3[0:32, 0] = DT * qb3[0]
    for comp in range(3):
        B3[32 + 32 * comp:64 + 32 * comp, 0] = SQRT_DT * zb3[comp]

    # broadcast matmul: qrep[32+32i+k] = out3s[k]
    BB = np.zeros((128, 128), np.float32)
    for k in range(CHUNKS):
        for comp in range(3):
            BB[k, 32 + 32 * comp + k] = 1.0

    # final reduction: col j sums the 3 components of chunk j
    TT = np.zeros((128, 32), np.float32)
    for j in range(CHUNKS):
        for comp in range(3):
            TT[32 + 32 * comp + j, j] = 1.0

    # initial y state, broadcast to full [128, 512] (q-slot rows zero)
    YINIT = np.zeros((128, FREE), np.float32)
    for comp in range(3):
        YINIT[32 + 32 * comp:64 + 32 * comp, :] = y0[comp]

    # per-core dW, transposed to [steps, comp, b_loc]; row block 0 (the q-slot
    # partitions) is zeros so a single full-tile DMA initializes everything
    dWt_cores = []
    for r in range(N_CORES):
        sl = dW[:N_STEPS, r * B_LOC:(r + 1) * B_LOC, :]     # [N_STEPS, B_loc, 3]
        t = np.zeros((N_STEPS, 4 * B_LOC), np.float32)
        t[:, B_LOC:] = sl.transpose(0, 2, 1).reshape(N_STEPS, 3 * B_LOC)
        dWt_cores.append(t)

    return dict(
        W1S=W1S, CB=CB, W2=W2, B2=B2, W3S=W3S, B3=B3, BB=BB, TT=TT,
        YINIT=YINIT, dWt_cores=dWt_cores, Y0=Y0,
    )


def _split_sync_waits(bir: dict) -> dict:
    """Walrus in this toolchain accepts only ~1 sync wait per instruction.
    Hoist extra waits onto standalone EventSemaphore instructions inserted
    just before, on the same engine (waits-only, so semantics unchanged)."""
    n = 0
    for fn in bir.get("functions", []):
        for bb in fn.get("blocks", []):
            out = []
            for ins in bb.get("instructions", []):
                si = ins.get("sync_info")
                waits = (si or {}).get("on_wait") or []
                if len(waits) > 1:
                    for w in waits[:-1]:
                        n += 1
                        out.append({
                            "engine": ins["engine"],
                            "ins": [],
                            "outs": [],
                            "name": f"bsdewait{n}_{ins['name']}",
                            "opcode": "EventSemaphore",
                            "debug": ins.get("debug", 0),
                            "sync_info": {"on_update": [], "on_wait": [w]},
                        })
                    si["on_wait"] = [waits[-1]]
                out.append(ins)
            bb["instructions"] = out
    return bir


def _install_ldw_opt():
    """walrus is invoked with --enable-ldw-opt=false; flip it on so repeated/
    adjacent stationary loads are optimized (gated by BSDE_LDW_OPT)."""
    from concourse import bass_utils
    if os.environ.get("BSDE_LDW_OPT", "0") != "1":
        return
    if getattr(bass_utils, "_bsde_ldwopt_installed", False):
        return
    orig = bass_utils.run_command

    def wrapped(cmd, **kw):
        if isinstance(cmd, list):
            cmd = ["--enable-ldw-opt=true" if c == "--enable-ldw-opt=false" else c
                   for c in cmd]
        return orig(cmd, **kw)

    bass_utils.run_command = wrapped
    bass_utils._bsde_ldwopt_installed = True


def _install_wait_splitter():
    import json as _json
    from concourse import bass2jax, bass_utils
    if getattr(bass_utils, "_bsde_split_installed", False):
        return
    orig = bass_utils.compile_bir_kernel

    def wrapped(bir_json, tmpdir, neff_name="file.neff"):
        bir = _json.loads(bir_json)
        _split_sync_waits(bir)
        return orig(_json.dumps(bir).encode(), tmpdir, neff_name)

    bass_utils.compile_bir_kernel = wrapped
    bass2jax.compile_bir_kernel = wrapped
    bass_utils._bsde_split_installed = True


def build_program():
    """Build the Bass program (same for all cores). Returns (nc, meta)."""
    from concourse import bass, mybir, tile

    f32 = mybir.dt.float32
    f32r = mybir.dt.float32r
    hdt = f32 if MM_HID_F32 else mybir.dt.bfloat16
    Alu = mybir.AluOpType
    Act = mybir.ActivationFunctionType

    def R(ap):
        # reinterpret fp32 data as float32r for full-rate PE streaming
        return ap.bitcast(f32r)

    def H(ap):
        # hidden-path operand: native bf16, or f32->f32r bitcast in fallback
        return R(ap) if hdt == f32 else ap

    nc = bass.Bass("TRN2", target_bir_lowering=False, debug=False)

    # --- dram I/O ---
    d_dWt = nc.dram_tensor("dWt", [N_STEPS, 4 * B_LOC], f32, kind="ExternalInput").ap()
    d_W1S = nc.dram_tensor("W1S", [128, CHUNKS * 128], f32r, kind="ExternalInput").ap()
    d_CB = nc.dram_tensor("CB", [128, N_STEPS], f32, kind="ExternalInput").ap()
    d_W2 = nc.dram_tensor("W2", [128, 128], f32, kind="ExternalInput").ap()
    d_B2 = nc.dram_tensor("B2", [128, 1], f32, kind="ExternalInput").ap()
    d_W3S = nc.dram_tensor("W3S", [128, CHUNKS * 128], f32, kind="ExternalInput").ap()
    d_B3 = nc.dram_tensor("B3", [128, 1], f32, kind="ExternalInput").ap()
    d_BB = nc.dram_tensor("BB", [128, 128], f32r, kind="ExternalInput").ap()
    d_TT = nc.dram_tensor("TT", [128, 32], f32r, kind="ExternalInput").ap()
    d_YI = nc.dram_tensor("YINIT", [128, FREE], f32r, kind="ExternalInput").ap()
    d_SC = nc.dram_tensor("SCAL", [4, 1], f32, kind="ExternalInput").ap()  # [Y0; -0.5/dt; a; b]
    d_res = nc.dram_tensor("res", [32, 1], f32, kind="ExternalOutput").ap()

    a_coef = 0.1 * SQRT_DT
    b_coef = 0.2 * SQRT_DT

    with tile.TileContext(nc) as tc:
        with (
            tc.tile_pool(name="consts", bufs=1) as consts,
            tc.tile_pool(name="state", bufs=1) as state,
            tc.tile_pool(name="h1p", bufs=3) as h1pool,
            tc.tile_pool(name="h2p", bufs=6) as h2pool,
            tc.tile_pool(name="tmp", bufs=1) as tmp,
            tc.tile_pool(name="dwp", bufs=8) as dwp,
            tc.tile_pool(name="ps1", bufs=2, space="PSUM") as ps1,
            tc.tile_pool(name="ps2", bufs=3, space="PSUM") as ps2,
            tc.tile_pool(name="ps3", bufs=1, space="PSUM") as ps3,
        ):
            # ---- load constants into SBUF ----
            def load_const(name, dram_ap, shape, dt_):
                t = consts.tile(shape, dt_, tag=name)
                nc.gpsimd.dma_start(t[:], dram_ap)
                return t

            W1S = load_const("W1S", d_W1S, [128, CHUNKS * 128], f32r)
            CB = load_const("CB", d_CB, [128, N_STEPS], f32)
            W2 = load_const("W2", d_W2, [128, 128], f32)
            B2 = load_const("B2", d_B2, [128, 1], f32)
            W3S = load_const("W3S", d_W3S, [128, CHUNKS * 128], f32)
            B3 = load_const("B3", d_B3, [128, 1], f32)
            BB = load_const("BB", d_BB, [128, 128], f32r)
            TT = load_const("TT", d_TT, [128, 32], f32r)

            if hdt == f32:
                W2m, W3Sm = W2, W3S
            else:
                W2m = consts.tile([128, 128], hdt, tag="W2m", name="W2m")
                nc.vector.tensor_copy(W2m[:], W2[:])
                W3Sm = consts.tile([128, CHUNKS * 128], hdt, tag="W3Sm", name="W3Sm")
                nc.vector.tensor_copy(W3Sm[:], W3S[:])

            # ---- persistent state ----
            y_pl = state.tile([128, FREE], f32r, tag="y_pl", name="y_pl")
            nc.gpsimd.dma_start(y_pl[:], d_YI)
            accA = state.tile([32, FREE], f32, tag="accA", name="accA")
            nc.vector.memset(accA[:], 0.0)
            accP = state.tile([128, FREE], f32, tag="accP", name="accP")
            nc.vector.memset(accP[:], 0.0)
            out3s = state.tile([128, FREE], f32r, tag="out3s", name="out3s")

            # ---- time loop ----
            # Per step: 16 chunk-PAIRS. L1 writes a 2-bank-wide psum tile per
            # pair, evacuated by ONE wide ACT op (halves ACT op-count
            # overhead). PE emission is software-pipelined: L1(pair j),
            # L2(pair j-1), L3(pair j-2) so the PE never waits on an evac.
            # SBUF-only elementwise work runs on the otherwise-idle GPSIMD.
            NP = CHUNKS // 2
            n_loop = N_STEPS if LOOP_STEPS is None else LOOP_STEPS
            for n in range(n_loop):
                dw_t = dwp.tile([128, FREE], f32, tag="dw", name="dw")
                nc.gpsimd.dma_start(dw_t[:, :], d_dWt[n, :])

                p3 = ps3.tile([128, FREE], f32, tag="p3", name="p3")
                cb_n = CB[:, n:n + 1]

                h1w_l = [None] * NP
                h2_l = [None] * CHUNKS
                for j in range(NP + 2):
                    # diffusion path (needs only y_n, dW_n): emitted mid-loop
                    # so it does not sit at the head of any engine queue and
                    # delay the first psum evacuations.
                    if j == 3:
                        th = tmp.tile([128, FREE], f32, tag="th", name="th")
                        nc.scalar.activation(th[:], y_pl[:], Act.Tanh)
                    elif j == 4:
                        t1 = tmp.tile([128, FREE], f32, tag="t1", name="t1")
                        nc.vector.tensor_scalar(t1[:], th[:], a_coef, b_coef,
                                                Alu.mult, Alu.add)
                    elif j == 5:
                        t2 = tmp.tile([128, FREE], f32, tag="t2", name="t2")
                        nc.gpsimd.tensor_tensor(t2[:], t1[:], dw_t[:], Alu.mult)
                    elif j == 9:
                        t3 = tmp.tile([128, FREE], f32, tag="t3", name="t3")
                        nc.vector.scalar_tensor_tensor(t3[:], y_pl[:], 1.0 - DT, t2[:],
                                                       Alu.mult, Alu.add)
                    if j < NP:
                        p1w = ps1.tile([128, 2 * FREE], f32, tag="p1", name="p1w")
                        nc.tensor.matmul(p1w[:, 0:FREE],
                                         W1S[:, (2 * j) * 128:(2 * j + 1) * 128], y_pl[:])
                        nc.tensor.matmul(p1w[:, FREE:2 * FREE],
                                         W1S[:, (2 * j + 1) * 128:(2 * j + 2) * 128], y_pl[:])
                        h1w = h1pool.tile([128, 2 * FREE], hdt, tag="h1w", name="h1w")
                        nc.scalar.activation(h1w[:], p1w[:], Act.Relu, bias=cb_n)
                        h1w_l[j] = h1w
                    jj = j - 1
                    if 0 <= jj < NP:
                        h1w = h1w_l[jj]
                        for t in range(2):
                            k = 2 * jj + t
                            p2 = ps2.tile([128, FREE], f32, tag="p2", name="p2")
                            nc.tensor.matmul(p2[:], H(W2m[:]),
                                             H(h1w[:, t * FREE:(t + 1) * FREE]))
                            h2 = h2pool.tile([128, FREE], hdt, tag="h2", name="h2")
                            if ACT_H2 and k % (CHUNKS // ACT_H2) == 0:
                                nc.scalar.activation(h2[:], p2[:], Act.Relu, bias=B2[:, 0:1])
                            else:
                                nc.vector.tensor_scalar(h2[:], p2[:], B2[:, 0:1], 0.0,
                                                        Alu.add, Alu.max)
                            h2_l[k] = h2
                    jk = j - 2
                    if 0 <= jk < NP:
                        for t in range(2):
                            k = 2 * jk + t
                            nc.tensor.matmul(
                                p3[:], H(W3Sm[:, k * 128:(k + 1) * 128]), H(h2_l[k][:]),
                                start=(k == 0), stop=(k == CHUNKS - 1),
                            )

                # out3s = psum3 + per-partition bias
                nc.vector.tensor_scalar(out3s[:], p3[:], B3[:, 0:1], None, Alu.add)

                # qrep = broadcast dt*q to component quarters (via PE).
                # pq reuses p3's bank: the WAR dep on out3s is exactly the
                # required ordering.
                pq = ps3.tile([128, FREE], f32, tag="p3", name="pq")
                nc.tensor.matmul(pq[:], BB[:], out3s[:])

                nc.vector.scalar_tensor_tensor(y_pl[:], t3[:], 0.0, pq[:], Alu.add, Alu.add)

                sqA = tmp.tile([32, FREE], f32, tag="sqA", name="sqA")
                nc.gpsimd.tensor_tensor(sqA[:], out3s[0:32, :], out3s[0:32, :], Alu.mult)
                nc.gpsimd.tensor_tensor(accA[:], accA[:], sqA[:], Alu.add)
                p6 = tmp.tile([128, FREE], f32, tag="p6", name="p6")
                nc.gpsimd.tensor_tensor(p6[:], out3s[:], dw_t[:], Alu.mult)
                nc.gpsimd.tensor_tensor(accP[:], accP[:], p6[:], Alu.add)

            # ---- final loss assembly ----
            ysq = tmp.tile([128, FREE], f32r, tag="ysq", name="ysq")
            nc.scalar.activation(ysq[:], y_pl[:], Act.Square)
            p_term = ps1.tile([32, FREE], f32, tag="p1", name="pterm")
            nc.tensor.matmul(p_term[:], TT[:], ysq[:])
            accPr = tmp.tile([128, FREE], f32r, tag="accPr", name="accPr")
            nc.vector.tensor_copy(accPr[:], accP[:])
            p_P = ps2.tile([32, FREE], f32, tag="p2", name="pP")
            nc.tensor.matmul(p_P[:], TT[:], accPr[:])

            Pg = tmp.tile([32, FREE], f32, tag="Pg", name="Pg")
            nc.vector.tensor_scalar(Pg[:], p_P[:], 0.0, None, Alu.add)
            Tg = tmp.tile([32, FREE], f32, tag="Tg", name="Tg")
            nc.vector.tensor_scalar(Tg[:], p_term[:], 0.0, None, Alu.add)
            D1 = tmp.tile([32, FREE], f32, tag="D1", name="D1")
            nc.vector.scalar_tensor_tensor(D1[:], accA[:], -0.5 / DT, Pg[:], Alu.mult, Alu.add)
            D2 = tmp.tile([32, FREE], f32, tag="D2", name="D2")
            nc.vector.scalar_tensor_tensor(D2[:], Tg[:], -1.0, D1[:], Alu.mult, Alu.add)
            # add Y0 (runtime input, broadcast from SCAL[0])
            sc = consts.tile([4, 1], f32, tag="SCAL", name="SCAL")
            nc.gpsimd.dma_start(sc[:], d_SC)
            y0b = consts.tile([32, 1], f32, tag="y0b", name="y0b")
            nc.gpsimd.dma_start(y0b[:], bass.AP(tensor=d_SC.tensor, offset=0, ap=[[0, 32], [1, 1]]))
            D3 = tmp.tile([32, FREE], f32, tag="D3", name="D3")
            nc.vector.tensor_scalar(D3[:], D2[:], y0b[:, 0:1], None, Alu.add)

            dsq = tmp.tile([32, FREE], f32, tag="dsq", name="dsq")
            res = state.tile([32, 1], f32, tag="res", name="res")
            nc.scalar.activation(dsq[:], D3[:], Act.Square, accum_out=res[:])
            nc.sync.dma_start(d_res, res[:])

    return nc


LAST_EXEC_NS = None
LAST_TRACE_DIR = None


def kernel(**inputs) -> np.ndarray:
    global LAST_EXEC_NS, LAST_TRACE_DIR
    from concourse.bass_utils import run_bass_kernel_spmd
    _install_wait_splitter()
    _install_ldw_opt()

    host = prep_host(inputs)

    nc = build_program()

    scal = np.array([[host["Y0"]], [-0.5 / DT], [0.1 * SQRT_DT], [0.2 * SQRT_DT]], np.float32)
    shared = dict(
        W1S=host["W1S"], CB=host["CB"], W2=host["W2"], B2=host["B2"],
        W3S=host["W3S"], B3=host["B3"], BB=host["BB"], TT=host["TT"],
        YINIT=host["YINIT"], SCAL=scal,
    )
    in_maps = []
    for r in range(N_CORES):
        m = dict(shared)
        m["dWt"] = host["dWt_cores"][r]
        in_maps.append(m)

    trace = os.environ.get("BSDE_TRACE", "0") == "1"
    kw = {}
    if trace:
        kw["trace"] = True
        kw["tmpdir"] = os.environ.get("BSDE_TRACE_DIR") or None
    out = run_bass_kernel_spmd(nc, in_maps, list(range(N_CORES)), **kw)
    LAST_EXEC_NS = getattr(out, "exec_time_ns", None)
    total = np.float64(0.0)
    for r in range(N_CORES):
        total += np.sum(out.results[r]["res"].astype(np.float64))
    return np.float32(total / BATCH)


def _build_in_maps(host):
    scal = np.array([[host["Y0"]], [-0.5 / DT], [0.1 * SQRT_DT], [0.2 * SQRT_DT]], np.float32)
    shared = dict(
        W1S=host["W1S"], CB=host["CB"], W2=host["W2"], B2=host["B2"],
        W3S=host["W3S"], B3=host["B3"], BB=host["BB"], TT=host["TT"],
        YINIT=host["YINIT"], SCAL=scal,
    )
    in_maps = []
    for r in range(N_CORES):
        m = dict(shared)
        m["dWt"] = host["dWt_cores"][r]
        in_maps.append(m)
    return in_maps


def timed_run(nc, in_maps, iters=7):
    """Mirror bass2jax.run_bass_via_pjrt's multi-core path, but keep inputs
    device-resident and time steady-state executions. Returns (results_core0,
    sorted wall times in ns per call)."""
    import time
    import jax
    from jax.sharding import Mesh, PartitionSpec, NamedSharding
    from jax.experimental.shard_map import shard_map
    from concourse import bass2jax, mybir

    bass2jax.install_neuronx_cc_hook()
    n_cores = N_CORES

    in_names, out_names, out_avals, zero_outs = [], [], [], []
    for alloc in nc.m.functions[0].allocations:
        if not isinstance(alloc, mybir.MemoryLocationSet):
            continue
        name = alloc.memorylocations[0].name
        if alloc.kind == "ExternalInput":
            in_names.append(name)
        elif alloc.kind == "ExternalOutput":
            out_names.append(name)
            shape = tuple(alloc.tensor_shape)
            dtype = mybir.dt.np(alloc.dtype)
            out_avals.append(jax.core.ShapedArray(shape, dtype))
            zero_outs.append(np.zeros(shape, dtype))
    n_params = len(in_names)
    n_outs = len(out_avals)
    all_names = in_names + out_names
    donate = tuple(range(n_params, n_params + n_outs))

    def _body(*args):
        outs = bass2jax._bass_exec_p.bind(
            *list(args),
            out_avals=tuple(out_avals),
            in_names=tuple(all_names),
            out_names=tuple(out_names),
            lowering_input_output_aliases=(),
            sim_require_finite=True,
            sim_require_nnan=True,
            nc=nc,
        )
        return tuple(outs)

    devices = jax.devices()[:n_cores]
    mesh = Mesh(np.asarray(devices), ("core",))
    in_specs = (PartitionSpec("core"),) * (n_params + n_outs)
    out_specs = (PartitionSpec("core"),) * len(out_names)
    sharded = jax.jit(
        shard_map(_body, mesh=mesh, in_specs=in_specs, out_specs=out_specs, check_rep=False),
        donate_argnums=donate,
        keep_unused=True,
    )
    concat_in = [
        np.concatenate([np.asarray(in_maps[c][nm]) for c in range(n_cores)], axis=0)
        for nm in in_names
    ]
    sh = NamedSharding(mesh, PartitionSpec("core"))
    dev_in = [jax.device_put(a, sh) for a in concat_in]
    concat_zeros = [np.zeros((n_cores * z.shape[0], *z.shape[1:]), z.dtype) for z in zero_outs]

    out = sharded(*dev_in, *concat_zeros)   # warm-up / compile
    jax.block_until_ready(out)
    times = []
    for _ in range(iters):
        zz = [np.zeros((n_cores * z.shape[0], *z.shape[1:]), z.dtype) for z in zero_outs]
        t0 = time.perf_counter_ns()
        out = sharded(*dev_in, *zz)
        jax.block_until_ready(out)
        times.append(time.perf_counter_ns() - t0)
    res0 = {
        nm: np.asarray(out[i]).reshape(n_cores, *out_avals[i].shape)
        for i, nm in enumerate(out_names)
    }
    return res0, sorted(times)


if __name__ == "__main__":
    rng = np.random.default_rng(0)
    fake = {
        "y0": rng.standard_normal(3).astype(np.float32),
        "Y0": np.zeros((1, 1), np.float32),
        "qW1": rng.standard_normal((4, 64)).astype(np.float32) * 0.5,
        "qb1": np.zeros(64, np.float32),
        "qW2": rng.standard_normal((64, 64)).astype(np.float32) * 0.12,
        "qb2": np.zeros(64, np.float32),
        "qW3": rng.standard_normal((64, 1)).astype(np.float32) * 0.12,
        "qb3": np.zeros(1, np.float32),
        "zW1": rng.standard_normal((4, 64)).astype(np.float32) * 0.5,
        "zb1": np.zeros(64, np.float32),
        "zW2": rng.standard_normal((64, 64)).astype(np.float32) * 0.12,
        "zb2": np.zeros(64, np.float32),
        "zW3": rng.standard_normal((64, 3)).astype(np.float32) * 0.12,
        "zb3": np.zeros(3, np.float32),
        "dW": rng.standard_normal((N_STEPS, BATCH, 3)).astype(np.float32),
    }
    print(kernel(**fake))

